# revision 10
# baseline (speedup 1.0000x reference)
# Multi-head attention (B=2, S=4096, D=512, H=8) on 8 Trainium2 NeuronCores.
#
# Sharding: core c handles batch b=c//4 and query rows [(c%4)*1024, (c%4+1)*1024).
# Each core computes K/V projections for its full batch element (duplicated
# across the 4 cores sharing a batch; avoids all cross-core communication),
# Q for its own slice, full 8-head attention for its query rows, and the
# output projection for its rows. The full output is the disjoint
# concatenation of the 8 per-core results.
#
# On-core dataflow (all matmuls bf16 with fp32 PSUM accumulation):
#   x,W --SWDGE cast--> bf16 DRAM --DMA-transpose--> x^T, W^T in SBUF
#   Q^T = Wq^T-tiles x x^T   K^T = Wk^T-tiles x x^T   V = x^T-tiles x Wv^T
#   per (q-chunk 512, head-pair): for each k-tile of 128:
#     S^T[k,q] = K^T x Q^T     (two heads row-packed in the PE array)
#     P^T = exp(S^T * scale)   (ACT engine, PSUM->SBUF, scale via free affine)
#     O^T[dh,q] += V-tile^T x P^T ; l[q] += ones^T x P^T  (column-packed)
#   O^T normalized by 1/l (DVE; l is produced broadcast across partitions)
#   y = O^T-tiles x Wout^T + b  (consumes O^T directly as stationary operand)
import sys

if "/opt/trn_rl_repo" not in sys.path:
    sys.path.insert(0, "/opt/trn_rl_repo")

import numpy as np

B = 2
S = 4096
DIM = 512
H = 8
DH = DIM // H
SCALE = DH**-0.5
N_CORES = 8
QLOC = S // 4  # query rows per core
N_KT = S // 128  # k tiles of 128
N_DT = DIM // 128  # feature-dim tiles of 128

_CACHE = {}


def _build_program():
    from contextlib import ExitStack

    from concourse import bacc, mybir, tile

    f32 = mybir.dt.float32
    bf16 = mybir.dt.bfloat16
    Exp = mybir.ActivationFunctionType.Exp

    nc = bacc.Bacc("TRN2", target_bir_lowering=False, debug=False)

    x_full = nc.dram_tensor("x_full", [S, DIM], f32, kind="ExternalInput")
    x_q = nc.dram_tensor("x_q", [QLOC, DIM], f32, kind="ExternalInput")
    w_qkv = nc.dram_tensor("w_qkv", [3 * DIM, DIM], f32, kind="ExternalInput")
    w_out = nc.dram_tensor("w_out", [DIM, DIM], f32, kind="ExternalInput")
    b_out = nc.dram_tensor("b_out", [1, DIM], f32, kind="ExternalInput")
    y_out = nc.dram_tensor("y", [QLOC, DIM], f32, kind="ExternalOutput")

    x_bf = nc.dram_tensor("x_bf", [S, DIM], bf16)
    xq_bf = nc.dram_tensor("xq_bf", [QLOC, DIM], bf16)
    wqkv_bf = nc.dram_tensor("wqkv_bf", [3 * DIM, DIM], bf16)
    wout_bf = nc.dram_tensor("wout_bf", [DIM, DIM], bf16)

    with tile.TileContext(nc) as tc, ExitStack() as ctx:
        consts = ctx.enter_context(tc.tile_pool(name="consts", bufs=1))
        wp = ctx.enter_context(tc.tile_pool(name="wp", bufs=1))
        big = ctx.enter_context(tc.tile_pool(name="big", bufs=1))
        xtp = ctx.enter_context(tc.tile_pool(name="xtp", bufs=2))
        ptp = ctx.enter_context(tc.tile_pool(name="ptp", bufs=2))
        otp = ctx.enter_context(tc.tile_pool(name="otp", bufs=2))
        rbp = ctx.enter_context(tc.tile_pool(name="rbp", bufs=2))
        ysp = ctx.enter_context(tc.tile_pool(name="ysp", bufs=2))
        pp = ctx.enter_context(tc.tile_pool(name="pp", bufs=2, space="PSUM"))
        sp = ctx.enter_context(tc.tile_pool(name="sp", bufs=1, space="PSUM"))
        op = ctx.enter_context(tc.tile_pool(name="op", bufs=1, space="PSUM"))

        # --- constants ---
        ones_sb = consts.tile([128, 64], bf16, tag="ones")
        nc.gpsimd.memset(ones_sb[:], 1.0)
        # zeros: lhsT/rhs of the accumulation-group bracket matmuls (see below)
        zeros_sb = consts.tile([128, 512], bf16, tag="zeros")
        nc.gpsimd.memset(zeros_sb[:], 0.0)
        bias_sb = consts.tile([128, DIM], f32, tag="bias")
        nc.gpsimd.dma_start(out=bias_sb[:], in_=b_out.ap().broadcast_to([128, DIM]))

        # --- f32 -> bf16 casts: HWDGE load -> DVE cast -> HWDGE store ---
        # (SWDGE DRAM->DRAM cast DMAs serialize on the GPSIMD engine; this
        # path keeps the cast on the otherwise-idle DVE + fast HWDGE queues.)
        castp = ctx.enter_context(tc.tile_pool(name="castp", bufs=2))
        cast_n = [0]

        def cast_chunk(dst_dram, src_dram, r0, rows):
            a = rows // 128
            cast_n[0] += 1
            xf = castp.tile([128, a, DIM], f32, tag="castf", name=f"castf{cast_n[0]}")
            nc.sync.dma_start(
                out=xf[:],
                in_=src_dram.ap()[r0 : r0 + rows, :].rearrange(
                    "(a p) d -> p a d", p=128
                ),
            )
            xb = castp.tile([128, a, DIM], bf16, tag="castb", name=f"castb{cast_n[0]}")
            nc.vector.tensor_copy(xb[:], xf[:])
            nc.sync.dma_start(
                out=dst_dram.ap()[r0 : r0 + rows, :].rearrange(
                    "(a p) d -> p a d", p=128
                ),
                in_=xb[:],
            )

        for ec in range(3):
            cast_chunk(wqkv_bf, w_qkv, ec * 512, 512)
        cast_chunk(wout_bf, w_out, 0, 512)
        for qc in range(QLOC // 512):
            cast_chunk(xq_bf, x_q, qc * 512, 512)

        # --- W^T via DMA transpose: wqkvT[d, e] layout [128, dt, 1536] ---
        wqkvT = wp.tile([128, N_DT, 3 * DIM], bf16, tag="wqkvT")
        for dt in range(N_DT):
            for ec in range(3):
                nc.sync.dma_start(
                    out=wqkvT[:, dt, ec * 512 : (ec + 1) * 512],
                    in_=wqkv_bf.ap()[ec * 512 : (ec + 1) * 512, dt * 128 : (dt + 1) * 128],
                    transpose=True,
                )
        woutT = wp.tile([128, N_DT, DIM], bf16, tag="woutT")
        for dt in range(N_DT):
            nc.sync.dma_start(
                out=woutT[:, dt, :],
                in_=wout_bf.ap()[:, dt * 128 : (dt + 1) * 128],
                transpose=True,
            )

        # --- persistent per-core tensors ---
        KT = big.tile([128, N_DT, S], bf16, tag="KT")  # K^T: part=(e-512)%128, [et, s]
        V = big.tile([128, S // 128, DIM], bf16, tag="V")  # V: part=s%128, [s-tile, e]
        QT = big.tile([128, N_DT, QLOC], bf16, tag="QT")  # Q^T: part=e%128, [et, q]

        # --- Q^T projection (own query slice) ---
        for qc in range(QLOC // 512):
            xqT = xtp.tile([128, N_DT, 512], bf16, tag="xT")
            for dt in range(N_DT):
                nc.sync.dma_start(
                    out=xqT[:, dt, :],
                    in_=xq_bf.ap()[qc * 512 : (qc + 1) * 512, dt * 128 : (dt + 1) * 128],
                    transpose=True,
                )
            for et in range(N_DT):
                ps = pp.tile([128, 512], f32, tag="proj")
                for dt in range(N_DT):
                    nc.tensor.matmul(
                        ps[:],
                        wqkvT[:, dt, et * 128 : (et + 1) * 128],
                        xqT[:, dt, :],
                        start=(dt == 0),
                        stop=(dt == N_DT - 1),
                    )
                nc.vector.tensor_copy(QT[:, et, qc * 512 : (qc + 1) * 512], ps[:])

        # --- attention pair machinery ---
        ot_state = {}

        def pair_begin(qc, j):
            # The two packed heads accumulate into disjoint partition halves of
            # one PSUM bank. Hardware tracks has_written per element, but the
            # start/stop accumulation-group flags act on the whole bank region,
            # so open each bank with a single full-128-partition zeroing matmul
            # (and close it symmetrically in pair_end) — exactly one
            # accumulation group per bank.
            ot = op.tile([128, 512], f32, tag="ot")
            lt = op.tile([128, 512], f32, tag="lt")
            for t in (ot, lt):
                nc.tensor.matmul(
                    t[:], zeros_sb[:, 0:128], zeros_sb[:], start=True, stop=False
                )
            ot_state[(qc, j)] = (ot, lt)

        def pair_groups(qc, j, groups, OT):
            ot, lt = ot_state[(qc, j)]
            q_sl = slice(qc * 512, (qc + 1) * 512)
            for g in groups:
                sA = sp.tile([128, 2, 512], f32, tag="stA")
                sB = sp.tile([128, 2, 512], f32, tag="stB")
                for u in range(2):
                    kt = 2 * g + u
                    k_sl = slice(kt * 128, (kt + 1) * 128)
                    nc.tensor.matmul(
                        sA[:, u, :], KT[0:64, j, k_sl], QT[0:64, j, q_sl],
                        start=True, stop=True, tile_position=(0, 0),
                    )
                    nc.tensor.matmul(
                        sB[:, u, :], KT[64:128, j, k_sl], QT[64:128, j, q_sl],
                        start=True, stop=True, tile_position=(64, 0),
                    )
                pA = ptp.tile([128, 2, 512], bf16, tag="ptA")
                pB = ptp.tile([128, 2, 512], bf16, tag="ptB")
                nc.scalar.activation(out=pA[:], in_=sA[:], func=Exp, scale=float(SCALE))
                nc.scalar.activation(out=pB[:], in_=sB[:], func=Exp, scale=float(SCALE))
                for u in range(2):
                    kt = 2 * g + u
                    hA, hB = 2 * j, 2 * j + 1
                    nc.tensor.matmul(
                        ot[0:64, :], V[:, kt, hA * DH : (hA + 1) * DH], pA[:, u, :],
                        start=False, stop=False, tile_position=(0, 0),
                    )
                    nc.tensor.matmul(
                        ot[64:128, :], V[:, kt, hB * DH : (hB + 1) * DH], pB[:, u, :],
                        start=False, stop=False, tile_position=(0, 64),
                    )
                    nc.tensor.matmul(
                        lt[0:64, :], ones_sb[:, 0:64], pA[:, u, :],
                        start=False, stop=False, tile_position=(0, 0),
                    )
                    nc.tensor.matmul(
                        lt[64:128, :], ones_sb[:, 0:64], pB[:, u, :],
                        start=False, stop=False, tile_position=(0, 64),
                    )

        def pair_end(qc, j, OT):
            ot, lt = ot_state.pop((qc, j))
            for t in (ot, lt):  # close the bank's accumulation group (adds zeros)
                nc.tensor.matmul(
                    t[:], zeros_sb[:, 0:128], zeros_sb[:], start=False, stop=True
                )
            rb = rbp.tile([128, 512], f32, tag="rb")
            nc.vector.reciprocal(out=rb[0:64, :], in_=lt[0:64, :])
            nc.vector.reciprocal(out=rb[64:128, :], in_=lt[64:128, :])
            nc.vector.tensor_mul(OT[0:64, j, :], ot[0:64, :], rb[0:64, :])
            nc.vector.tensor_mul(OT[64:128, j, :], ot[64:128, :], rb[64:128, :])

        def emit_y(qc, OT):
            for st in range(4):
                yp = pp.tile([128, 512], f32, tag="proj")
                for dt in range(N_DT):
                    nc.tensor.matmul(
                        yp[:],
                        OT[:, dt, st * 128 : (st + 1) * 128],
                        woutT[:, dt, :],
                        start=(dt == 0),
                        stop=(dt == N_DT - 1),
                    )
                ys = ysp.tile([128, 512], f32, tag="ysb")
                nc.vector.tensor_add(ys[:], yp[:], bias_sb[:])
                nc.sync.dma_start(
                    out=y_out.ap()[qc * 512 + st * 128 : qc * 512 + (st + 1) * 128, :],
                    in_=ys[:],
                )

        OT_tiles = {}
        OT_tiles[0] = otp.tile([128, N_DT, 512], bf16, tag="OT", name="OT0")

        # --- K/V projection interleaved with the first attention pair ---
        pair_begin(0, 0)
        cast_chunk(x_bf, x_full, 0, 512)
        cast_chunk(x_bf, x_full, 512, 512)
        for sc in range(S // 512):
            if sc + 2 < S // 512:
                cast_chunk(x_bf, x_full, (sc + 2) * 512, 512)
            xT = xtp.tile([128, N_DT, 512], bf16, tag="xT")
            for dt in range(N_DT):
                nc.sync.dma_start(
                    out=xT[:, dt, :],
                    in_=x_bf.ap()[sc * 512 : (sc + 1) * 512, dt * 128 : (dt + 1) * 128],
                    transpose=True,
                )
            for et in range(N_DT):
                ps = pp.tile([128, 512], f32, tag="proj")
                for dt in range(N_DT):
                    nc.tensor.matmul(
                        ps[:],
                        wqkvT[:, dt, DIM + et * 128 : DIM + (et + 1) * 128],
                        xT[:, dt, :],
                        start=(dt == 0),
                        stop=(dt == N_DT - 1),
                    )
                nc.vector.tensor_copy(KT[:, et, sc * 512 : (sc + 1) * 512], ps[:])
            for a in range(4):
                ps = pp.tile([128, 512], f32, tag="proj")
                for dt in range(N_DT):
                    nc.tensor.matmul(
                        ps[:],
                        xT[:, dt, a * 128 : (a + 1) * 128],
                        wqkvT[:, dt, 2 * DIM : 3 * DIM],
                        start=(dt == 0),
                        stop=(dt == N_DT - 1),
                    )
                nc.vector.tensor_copy(V[:, sc * 4 + a, :], ps[:])
            # attention on pair (qc=0, j=0) for k-groups now available
            pair_groups(0, 0, [2 * sc, 2 * sc + 1], OT_tiles[0])
        pair_end(0, 0, OT_tiles[0])

        # --- remaining pairs ---
        OT_tiles[1] = otp.tile([128, N_DT, 512], bf16, tag="OT", name="OT1")
        for qc, j in [(1, 0), (0, 1), (1, 1), (0, 2), (1, 2), (0, 3), (1, 3)]:
            pair_begin(qc, j)
            pair_groups(qc, j, list(range(N_KT // 2)), OT_tiles[qc])
            pair_end(qc, j, OT_tiles[qc])
            if (qc, j) == (0, 3):
                emit_y(0, OT_tiles[0])
        emit_y(1, OT_tiles[1])

    nc.compile()
    return nc


def _get_nc():
    if "nc" not in _CACHE:
        _CACHE["nc"] = _build_program()
    return _CACHE["nc"]


def sim_time_estimate():
    """CoreSim cost-model span for one core with zero-filled inputs."""
    from concourse.bass_interp import CoreSim

    nc = _get_nc()
    sim = CoreSim(nc, publish_trace=False)
    sim.tensor("x_full")[:] = 0
    sim.tensor("x_q")[:] = 0
    sim.tensor("w_qkv")[:] = 0
    sim.tensor("w_out")[:] = 0
    sim.tensor("b_out")[:] = 0
    sim.simulate()
    return int(sim.time)


def kernel(x, w_qkv, w_out, b_out):
    from concourse.bass_utils import run_bass_kernel_spmd

    nc = _get_nc()
    x = np.asarray(x, dtype=np.float32)
    w_qkv = np.ascontiguousarray(np.asarray(w_qkv, dtype=np.float32))
    w_out = np.ascontiguousarray(np.asarray(w_out, dtype=np.float32))
    b_out = np.ascontiguousarray(np.asarray(b_out, dtype=np.float32)).reshape(1, DIM)

    in_maps = []
    for c in range(N_CORES):
        b = c // 4
        qo = (c % 4) * QLOC
        in_maps.append(
            {
                "x_full": np.ascontiguousarray(x[b]),
                "x_q": np.ascontiguousarray(x[b, qo : qo + QLOC]),
                "w_qkv": w_qkv,
                "w_out": w_out,
                "b_out": b_out,
            }
        )
    res = run_bass_kernel_spmd(nc, in_maps, list(range(N_CORES)))
    y = np.empty((B, S, DIM), dtype=np.float32)
    for c in range(N_CORES):
        b = c // 4
        qo = (c % 4) * QLOC
        y[b, qo : qo + QLOC] = res.results[c]["y"]
    return y
